# revision 12
# baseline (speedup 1.0000x reference)
"""nn_Attention — tensor-parallel causal attention on 8 TRN2 NeuronCores.

Contract: kernel(**inputs) takes the FULL unsharded inputs of the reference
(hidden_states (2,2048,2048) f32, c_attn_w (2048,6144), c_attn_b (6144,),
c_proj_w (2048,2048), c_proj_b (2048,)) and returns the full (2,2048,2048)
f32 output.

Sharding: batch x head-group tensor parallelism. Core c -> batch c//4,
head-group c%4 (4 of the 16 heads). Host-side prep per core: slice + cast
weights to bf16, and pre-transpose the batch's activations to xT [E, S]
bf16 so the contraction dim lands on SBUF partitions with plain big-packet
DMAs (no device-side cast/transpose at all). The host gather sums the 4
c_proj partials per batch and adds the c_proj bias.

Device pipeline (per core, fully fused per 512-row chunk; all matmuls bf16
with fp32 PSUM accumulation; NOTHING runs on GpSimd — its Q7 activity
power-throttles the PE to 50% util):
  - xT chunk tiles + weight column-groups stream on two parallel DMA
    queues (sync / scalar),
  - qkvT chunk = (Wqkv_slice^T x^T) + b; v kept natural via swapped
    operands,
  - per head: scoresT j-blocks = kT^T qT with causal trimming (diagonal
    blocks compute only the live N=512-128*dt columns), exp on ScalarE,
    triangular mask multiply on VectorE, attention accumulated transposed
    (outT += v_nat @ expT); row sums accumulated across j-blocks in bf16 on
    VectorE (2x DVE mode) and partition-reduced by ONE ones-matmul per
    (head, chunk) instead of one per j-block,
  - c_proj of chunk sc-1 interleaved between attention heads of chunk sc to
    cover ScalarE exp latency and kill the output tail.
"""

import os
import sys

for _p in ("/opt/trn_rl_repo", "/root/.axon_site/_ro/trn_rl_repo"):
    if os.path.isdir(_p) and _p not in sys.path:
        sys.path.append(_p)

from contextlib import ExitStack

import numpy as np

import concourse.bass as bass
import concourse.tile as tile
from concourse import bacc, mybir
from concourse.bass_utils import run_bass_kernel_spmd

F32 = mybir.dt.float32
BF16 = mybir.dt.bfloat16
P = 128
CHUNK = 512
DIAG = CHUNK // P

S, E, NHEAD = 2048, 2048, 16
BATCH = 2
H = 4            # heads per core
NJ = 3 * H       # j-blocks in wqkv slice
NQK = 2 * H      # transposed-projection j-blocks (q,k only)
EB = E // P
SC = S // CHUNK
SB = S // P
EC = E // CHUNK
N_CORES = 8


def _emit(nc):
    scale = 1.0 / float(np.sqrt(P))

    xT = nc.dram_tensor("xT", [E, S], BF16, kind="ExternalInput").ap()
    wqkv = nc.dram_tensor("wqkv", [E, NJ * P], BF16, kind="ExternalInput").ap()
    bqkv = nc.dram_tensor("bqkv", [P, NJ], F32, kind="ExternalInput").ap()
    wproj = nc.dram_tensor("wproj", [H * P, E], BF16, kind="ExternalInput").ap()
    tri = nc.dram_tensor("tri", [P, CHUNK], BF16, kind="ExternalInput").ap()
    ones = nc.dram_tensor("ones", [P, P], BF16, kind="ExternalInput").ap()
    y = nc.dram_tensor("y", [S, E], F32, kind="ExternalOutput").ap()

    xT_t = xT.rearrange("(eb p) s -> p eb s", p=P)
    wqkv_t = wqkv.rearrange("(gb e p) j -> gb p e j", p=P, e=4)
    wproj_t = wproj.rearrange("(hb p) e -> p hb e", p=P)

    with tile.TileContext(nc) as tc, ExitStack() as ctx:
        const = ctx.enter_context(tc.tile_pool(name="const", bufs=1))
        wq_pool = ctx.enter_context(tc.tile_pool(name="wq", bufs=1))
        wp_pool = ctx.enter_context(tc.tile_pool(name="wp", bufs=1))
        kT_pool = ctx.enter_context(tc.tile_pool(name="kT", bufs=1))
        vnat_pool = ctx.enter_context(tc.tile_pool(name="vnat", bufs=1))
        xTc_pool = ctx.enter_context(tc.tile_pool(name="xTc", bufs=2))
        qT_pool = ctx.enter_context(tc.tile_pool(name="qT", bufs=2))
        outT_pool = ctx.enter_context(tc.tile_pool(name="outT", bufs=2))
        exsum_pool = ctx.enter_context(tc.tile_pool(name="exsum", bufs=2))
        exp_pool = ctx.enter_context(tc.tile_pool(name="exp", bufs=6))
        yout_pool = ctx.enter_context(tc.tile_pool(name="yout", bufs=4))
        recip_pool = ctx.enter_context(tc.tile_pool(name="recip", bufs=2))
        mm_ps = ctx.enter_context(tc.tile_pool(name="mm_ps", bufs=2, space="PSUM"))
        sc2_ps = ctx.enter_context(tc.tile_pool(name="sc2_ps", bufs=2, space="PSUM"))
        out_ps_pool = ctx.enter_context(
            tc.tile_pool(name="out_ps", bufs=2, space="PSUM")
        )

        # consts + weights stream on the scalar-engine HWDGE queue, in
        # parallel with xT tiles on the sync queue. wqkv goes as 4 grouped
        # mega-tile DMAs (DMA issue rate, not bandwidth, limited startup).
        wq_grps = [
            wq_pool.tile([P, 4 * NJ * P], BF16, name=f"wqg{gb}") for gb in range(4)
        ]
        qk_c = NQK * P
        for gb in range(4):
            t = wq_grps[gb].rearrange("p (e j) -> p e j", e=4)
            nc.scalar.dma_start(t[:, :, :qk_c], wqkv_t[gb][:, :, :qk_c])
        bq_t = const.tile([P, NJ], F32)
        nc.scalar.dma_start(bq_t[:], bqkv[:])
        tri_t = const.tile([P, CHUNK], BF16)
        nc.scalar.dma_start(tri_t[:], tri[:])
        ones_t = const.tile([P, P], BF16)
        nc.scalar.dma_start(ones_t[:], ones[:])
        for gb in range(4):
            t = wq_grps[gb].rearrange("p (e j) -> p e j", e=4)
            nc.scalar.dma_start(t[:, :, qk_c:], wqkv_t[gb][:, :, qk_c:])
        wq_tiles = [
            wq_grps[eb // 4][:, (eb % 4) * NJ * P : (eb % 4 + 1) * NJ * P]
            for eb in range(EB)
        ]
        wp_all = wp_pool.tile([P, H * E], BF16, name="wpall")
        nc.scalar.dma_start(wp_all.rearrange("p (hb e) -> p hb e", hb=H), wproj_t)
        wp_tiles = [wp_all[:, hb * E : (hb + 1) * E] for hb in range(H)]

        kT = [kT_pool.tile([P, S], BF16, name=f"kT{h}") for h in range(H)]
        vnat = [vnat_pool.tile([P, H * P], BF16, name=f"vn{sb}") for sb in range(SB)]

        prev_outT = None  # outT tiles of the previous chunk, for c_proj
        prev_sc = None

        def emit_proj(sc_p, sb_local, outT_tiles):
            sb = sc_p * DIAG + sb_local
            for ec in range(EC):
                ps = mm_ps.tile([P, CHUNK], F32, name="mm")
                for h in range(H):
                    nc.tensor.matmul(
                        ps[:],
                        outT_tiles[h][:, sb_local * P : (sb_local + 1) * P],
                        wp_tiles[h][:, ec * CHUNK : (ec + 1) * CHUNK],
                        start=(h == 0),
                        stop=(h == H - 1),
                    )
                yo = yout_pool.tile([P, CHUNK], F32, name="yo")
                if ec % 2 == 0:
                    nc.vector.tensor_copy(yo[:], ps[:])
                else:
                    nc.scalar.copy(yo[:], ps[:])
                nc.sync.dma_start(
                    y[sb * P : (sb + 1) * P, ec * CHUNK : (ec + 1) * CHUNK], yo[:]
                )

        def load_chunk(sc_l):
            xTc = xTc_pool.tile([P, EB * CHUNK], BF16, name="xTc")
            xTv = xTc.rearrange("p (eb c) -> p eb c", c=CHUNK)
            nc.sync.dma_start(
                xTv[:], xT_t[:, :, sc_l * CHUNK : (sc_l + 1) * CHUNK]
            )
            return xTv

        xTv_next = load_chunk(0)
        for sc in range(SC):
            s0 = sc * CHUNK
            xTv = xTv_next
            if sc + 1 < SC:
                xTv_next = load_chunk(sc + 1)

            # ---- QKV projections for this chunk ----
            qTc = []
            for jb in range(NQK):
                ps = mm_ps.tile([P, CHUNK], F32, name="mm")
                for eb in range(EB):
                    nc.tensor.matmul(
                        ps[:],
                        wq_tiles[eb][:, jb * P : (jb + 1) * P],
                        xTv[:, eb],
                        start=(eb == 0),
                        stop=(eb == EB - 1),
                    )
                if jb < H:
                    t = qT_pool.tile([P, CHUNK], BF16, name=f"qT{jb}")
                    nc.vector.tensor_scalar_add(t[:], ps[:], bq_t[:, jb : jb + 1])
                    qTc.append(t)
                else:
                    nc.vector.tensor_scalar_add(
                        kT[jb - H][:, s0 : s0 + CHUNK], ps[:], bq_t[:, jb : jb + 1]
                    )
            for r in range(DIAG):
                ps = mm_ps.tile([P, H * P], F32, name="mm")
                for eb in range(EB):
                    nc.tensor.matmul(
                        ps[:],
                        xTv[:, eb, r * P : (r + 1) * P],
                        wq_tiles[eb][:, NQK * P : NJ * P],
                        start=(eb == 0),
                        stop=(eb == EB - 1),
                    )
                nc.scalar.copy(vnat[sc * DIAG + r][:], ps[:])

            # ---- attention for this chunk, proj of previous interleaved ----
            cur_outT = []
            for h in range(H):
                out_ps = out_ps_pool.tile([P, CHUNK], F32, name="outp")
                # diagonal blocks first (dt=0 is the full-width start=True
                # block), then the off-diagonal history blocks.
                seq = [(sc * DIAG + dt, dt) for dt in range(DIAG)] + [
                    (jb, None) for jb in range(sc * DIAG)
                ]
                last = len(seq) - 1
                es = exsum_pool.tile([P, CHUNK], BF16, name="exsum")
                for idx, (jb, dt) in enumerate(seq):
                    qoff = (dt or 0) * P
                    n = CHUNK - qoff
                    sps = mm_ps.tile([P, CHUNK], F32, name="mm")
                    nc.tensor.matmul(
                        sps[:, :n],
                        kT[h][:, jb * P : (jb + 1) * P],
                        qTc[h][:, qoff:CHUNK],
                        start=True,
                        stop=True,
                    )
                    ext = exp_pool.tile([P, CHUNK], BF16, name="ex")
                    nc.scalar.activation(
                        ext[:, :n],
                        sps[:, :n],
                        mybir.ActivationFunctionType.Exp,
                        scale=scale,
                    )
                    if dt is not None:
                        exm = exp_pool.tile([P, CHUNK], BF16, name="ex")
                        nc.vector.tensor_mul(exm[:, :n], ext[:, :n], tri_t[:, :n])
                        ext = exm
                    if idx == 0:
                        nc.vector.tensor_copy(es[:], ext[:])
                    else:
                        nc.vector.tensor_add(
                            es[:, qoff:CHUNK], es[:, qoff:CHUNK], ext[:, :n]
                        )
                    nc.tensor.matmul(
                        out_ps[:, qoff:CHUNK],
                        vnat[jb][:, h * P : (h + 1) * P],
                        ext[:, :n],
                        start=(idx == 0),
                        stop=(idx == last),
                        skip_group_check=True,
                    )
                # proj matmuls of the previous chunk run before this head's
                # sum-matmul, giving the VectorE exsum chain time to drain.
                if prev_outT is not None:
                    emit_proj(prev_sc, h, prev_outT)
                sum_ps = mm_ps.tile([P, CHUNK], F32, name="mm")
                nc.tensor.matmul(sum_ps[:], ones_t[:], es[:], start=True, stop=True)
                rc = recip_pool.tile([P, CHUNK], F32, name="rc")
                nc.vector.reciprocal_approx_fast(rc[:], sum_ps[:])
                oT = outT_pool.tile([P, CHUNK], BF16, name=f"oT{h}")
                nc.vector.tensor_mul(oT[:], out_ps[:], rc[:])
                cur_outT.append(oT)
            prev_outT = cur_outT
            prev_sc = sc

        for sb_local in range(DIAG):
            emit_proj(SC - 1, sb_local, prev_outT)
    return nc


_NC = None
LAST_RESULTS = None


def _get_nc():
    global _NC
    if _NC is None:
        nc = bacc.Bacc(
            "TRN2", target_bir_lowering=False, debug=False, num_devices=N_CORES
        )
        _emit(nc)
        nc.compile()
        _NC = nc
    return _NC


def _core_inputs(hidden_states, c_attn_w, c_attn_b, c_proj_w, core):
    import ml_dtypes

    bf16 = ml_dtypes.bfloat16
    b, g = core // 4, core % 4
    h0 = H * g
    cols = []
    for part in range(3):
        for h in range(h0, h0 + H):
            base = part * E + h * P
            cols.extend(range(base, base + P))
    cols = np.asarray(cols)
    wqkv = np.ascontiguousarray(c_attn_w[:, cols]).astype(bf16)
    bq = np.ascontiguousarray(c_attn_b[cols]).astype(np.float32)
    bq = bq.reshape(NJ, P).T.copy()
    wproj = np.ascontiguousarray(c_proj_w[h0 * P : (h0 + H) * P, :]).astype(bf16)
    ii = np.arange(CHUNK)[None, :]
    pp = np.arange(P)[:, None]
    tri = (pp <= ii).astype(bf16)
    ones = np.ones((P, P), dtype=bf16)
    xT = np.ascontiguousarray(hidden_states[b].astype(bf16).T)
    return {
        "xT": xT,
        "wqkv": wqkv,
        "bqkv": bq,
        "wproj": wproj,
        "tri": tri,
        "ones": ones,
    }


def kernel(hidden_states, c_attn_w, c_attn_b, c_proj_w, c_proj_b):
    global LAST_RESULTS
    hidden_states = np.asarray(hidden_states)
    c_attn_w = np.asarray(c_attn_w)
    c_attn_b = np.asarray(c_attn_b)
    c_proj_w = np.asarray(c_proj_w)
    c_proj_b = np.asarray(c_proj_b)

    nc = _get_nc()
    in_maps = [
        _core_inputs(hidden_states, c_attn_w, c_attn_b, c_proj_w, c)
        for c in range(N_CORES)
    ]
    res = run_bass_kernel_spmd(nc, in_maps, list(range(N_CORES)))
    LAST_RESULTS = res
    out = np.zeros((BATCH, S, E), dtype=np.float32)
    for c in range(N_CORES):
        out[c // 4] += res.results[c]["y"]
    out += c_proj_b.astype(np.float32)[None, None, :]
    return out


# revision 16
# speedup vs baseline: 1.0442x; 1.0442x over previous
"""nn_Attention — tensor-parallel causal attention on 8 TRN2 NeuronCores.

Contract: kernel(**inputs) takes the FULL unsharded inputs of the reference
(hidden_states (2,2048,2048) f32, c_attn_w (2048,6144), c_attn_b (6144,),
c_proj_w (2048,2048), c_proj_b (2048,)) and returns the full (2,2048,2048)
f32 output.

Sharding: batch x head-group tensor parallelism. Core c -> batch c//4,
head-group c%4 (4 of the 16 heads). Host-side prep per core: slice + cast
weights to bf16, and pre-transpose the batch's activations to xT [E, S]
bf16 so the contraction dim lands on SBUF partitions with plain big-packet
DMAs (no device-side cast/transpose at all). The host gather sums the 4
c_proj partials per batch and adds the c_proj bias.

Device pipeline (per core, fully fused per 512-row chunk; all matmuls bf16
with fp32 PSUM accumulation; NOTHING runs on GpSimd — its Q7 activity
power-throttles the PE to 50% util):
  - xT chunk tiles + weight column-groups stream on two parallel DMA
    queues (sync / scalar),
  - qkvT chunk = (Wqkv_slice^T x^T) + b; v kept natural via swapped
    operands,
  - per head: scoresT j-blocks = kT^T qT with causal trimming (diagonal
    blocks compute only the live N=512-128*dt columns), exp on ScalarE,
    triangular mask multiply on VectorE, attention accumulated transposed
    (outT += v_nat @ expT); row sums accumulated across j-blocks in bf16 on
    VectorE (2x DVE mode) and partition-reduced by ONE ones-matmul per
    (head, chunk) instead of one per j-block,
  - c_proj of chunk sc-1 interleaved between attention heads of chunk sc to
    cover ScalarE exp latency and kill the output tail.
"""

import os
import sys

for _p in ("/opt/trn_rl_repo", "/root/.axon_site/_ro/trn_rl_repo"):
    if os.path.isdir(_p) and _p not in sys.path:
        sys.path.append(_p)

from contextlib import ExitStack

import numpy as np

import concourse.bass as bass
import concourse.tile as tile
from concourse import bacc, mybir
from concourse.bass_utils import run_bass_kernel_spmd

F32 = mybir.dt.float32
BF16 = mybir.dt.bfloat16
P = 128
CHUNK = 512
DIAG = CHUNK // P

S, E, NHEAD = 2048, 2048, 16
BATCH = 2
H = 4            # heads per core
NJ = 3 * H       # j-blocks in wqkv slice
NQK = 2 * H      # transposed-projection j-blocks (q,k only)
EB = E // P
SC = S // CHUNK
SB = S // P
EC = E // CHUNK
N_CORES = 8


def _emit(nc):
    scale = 1.0 / float(np.sqrt(P))

    xT = nc.dram_tensor("xT", [E, S], BF16, kind="ExternalInput").ap()
    wqkv = nc.dram_tensor("wqkv", [E, NJ * P], BF16, kind="ExternalInput").ap()
    bqkv = nc.dram_tensor("bqkv", [P, NJ], F32, kind="ExternalInput").ap()
    wproj = nc.dram_tensor("wproj", [H * P, E], BF16, kind="ExternalInput").ap()
    tri4 = nc.dram_tensor("tri4", [P, 1280], BF16, kind="ExternalInput").ap()
    ones = nc.dram_tensor("ones", [P, P], BF16, kind="ExternalInput").ap()
    y = nc.dram_tensor("y", [S, E], F32, kind="ExternalOutput").ap()

    xT_t = xT.rearrange("(eb p) s -> p eb s", p=P)
    wqkv_t = wqkv.rearrange("(gb e p) j -> gb p e j", p=P, e=4)
    wproj_t = wproj.rearrange("(hb p) e -> p hb e", p=P)

    with tile.TileContext(nc) as tc, ExitStack() as ctx:
        const = ctx.enter_context(tc.tile_pool(name="const", bufs=1))
        wq_pool = ctx.enter_context(tc.tile_pool(name="wq", bufs=1))
        wp_pool = ctx.enter_context(tc.tile_pool(name="wp", bufs=1))
        kT_pool = ctx.enter_context(tc.tile_pool(name="kT", bufs=1))
        vnat_pool = ctx.enter_context(tc.tile_pool(name="vnat", bufs=1))
        xTc_pool = ctx.enter_context(tc.tile_pool(name="xTc", bufs=2))
        qT_pool = ctx.enter_context(tc.tile_pool(name="qT", bufs=2))
        outT_pool = ctx.enter_context(tc.tile_pool(name="outT", bufs=2))
        exsum_pool = ctx.enter_context(tc.tile_pool(name="exsum", bufs=2))
        exp_pool = ctx.enter_context(tc.tile_pool(name="exp", bufs=6))
        yout_pool = ctx.enter_context(tc.tile_pool(name="yout", bufs=4))
        recip_pool = ctx.enter_context(tc.tile_pool(name="recip", bufs=2))
        mm_ps = ctx.enter_context(tc.tile_pool(name="mm_ps", bufs=2, space="PSUM"))
        sc2_ps = ctx.enter_context(tc.tile_pool(name="sc2_ps", bufs=2, space="PSUM"))
        out_ps_pool = ctx.enter_context(
            tc.tile_pool(name="out_ps", bufs=2, space="PSUM")
        )

        # consts + weights stream on the scalar-engine HWDGE queue, in
        # parallel with xT tiles on the sync queue. wqkv goes as 4 grouped
        # mega-tile DMAs (DMA issue rate, not bandwidth, limited startup).
        wq_grps = [
            wq_pool.tile([P, 4 * NJ * P], BF16, name=f"wqg{gb}") for gb in range(4)
        ]
        qk_c = NQK * P
        for gb in range(4):
            t = wq_grps[gb].rearrange("p (e j) -> p e j", e=4)
            nc.scalar.dma_start(t[:, :, :qk_c], wqkv_t[gb][:, :, :qk_c])
        bq_t = const.tile([P, NJ], F32)
        nc.scalar.dma_start(bq_t[:], bqkv[:])
        tri4_t = const.tile([P, 1280], BF16)
        nc.scalar.dma_start(tri4_t[:], tri4[:])
        ones_t = const.tile([P, P], BF16)
        nc.scalar.dma_start(ones_t[:], ones[:])
        for gb in range(4):
            t = wq_grps[gb].rearrange("p (e j) -> p e j", e=4)
            nc.scalar.dma_start(t[:, :, qk_c:], wqkv_t[gb][:, :, qk_c:])
        wq_tiles = [
            wq_grps[eb // 4][:, (eb % 4) * NJ * P : (eb % 4 + 1) * NJ * P]
            for eb in range(EB)
        ]
        wp_all = wp_pool.tile([P, H * E], BF16, name="wpall")
        nc.scalar.dma_start(wp_all.rearrange("p (hb e) -> p hb e", hb=H), wproj_t)
        wp_tiles = [wp_all[:, hb * E : (hb + 1) * E] for hb in range(H)]

        kT = [kT_pool.tile([P, S], BF16, name=f"kT{h}") for h in range(H)]
        vnat = [vnat_pool.tile([P, H * P], BF16, name=f"vn{sb}") for sb in range(SB)]

        prev_outT = None  # outT tiles of the previous chunk, for c_proj
        prev_sc = None

        def emit_proj(sc_p, sb_local, outT_tiles):
            sb = sc_p * DIAG + sb_local
            for ec in range(EC):
                ps = mm_ps.tile([P, CHUNK], F32, name="mm")
                for h in range(H):
                    nc.tensor.matmul(
                        ps[:],
                        outT_tiles[h][:, sb_local * P : (sb_local + 1) * P],
                        wp_tiles[h][:, ec * CHUNK : (ec + 1) * CHUNK],
                        start=(h == 0),
                        stop=(h == H - 1),
                    )
                yo = yout_pool.tile([P, CHUNK], F32, name="yo")
                if ec % 2 == 0:
                    nc.vector.tensor_copy(yo[:], ps[:])
                else:
                    nc.scalar.copy(yo[:], ps[:])
                nc.sync.dma_start(
                    y[sb * P : (sb + 1) * P, ec * CHUNK : (ec + 1) * CHUNK], yo[:]
                )

        def load_chunk(sc_l):
            xTc = xTc_pool.tile([P, EB * CHUNK], BF16, name="xTc")
            xTv = xTc.rearrange("p (eb c) -> p eb c", c=CHUNK)
            nc.sync.dma_start(
                xTv[:], xT_t[:, :, sc_l * CHUNK : (sc_l + 1) * CHUNK]
            )
            return xTv

        xTv_next = load_chunk(0)
        for sc in range(SC):
            s0 = sc * CHUNK
            xTv = xTv_next
            if sc + 1 < SC:
                xTv_next = load_chunk(sc + 1)

            # ---- QKV projections for this chunk ----
            qTc = []
            for jb in range(NQK):
                ps = mm_ps.tile([P, CHUNK], F32, name="mm")
                for eb in range(EB):
                    nc.tensor.matmul(
                        ps[:],
                        wq_tiles[eb][:, jb * P : (jb + 1) * P],
                        xTv[:, eb],
                        start=(eb == 0),
                        stop=(eb == EB - 1),
                    )
                if jb < H:
                    t = qT_pool.tile([P, CHUNK], BF16, name=f"qT{jb}")
                    nc.vector.tensor_scalar_add(t[:], ps[:], bq_t[:, jb : jb + 1])
                    qTc.append(t)
                else:
                    nc.vector.tensor_scalar_add(
                        kT[jb - H][:, s0 : s0 + CHUNK], ps[:], bq_t[:, jb : jb + 1]
                    )
            for r in range(DIAG):
                ps = mm_ps.tile([P, H * P], F32, name="mm")
                for eb in range(EB):
                    nc.tensor.matmul(
                        ps[:],
                        xTv[:, eb, r * P : (r + 1) * P],
                        wq_tiles[eb][:, NQK * P : NJ * P],
                        start=(eb == 0),
                        stop=(eb == EB - 1),
                    )
                nc.scalar.copy(vnat[sc * DIAG + r][:], ps[:])

            # ---- attention for this chunk, proj of previous interleaved ----
            cur_outT = []
            for h in range(H):
                out_ps = out_ps_pool.tile([P, CHUNK], F32, name="outp")
                # diagonal blocks first (dt=0 is the full-width start=True
                # block), then the off-diagonal history blocks. Blocks are
                # processed in PAIRS packed back-to-back in one 2-bank PSUM
                # tile so ScalarE runs ONE exp per pair (halves the per-op
                # PSUM access overhead).
                seq = [(sc * DIAG + dt, dt) for dt in range(DIAG)] + [
                    (jb, None) for jb in range(sc * DIAG)
                ]
                last = len(seq) - 1
                es = exsum_pool.tile([P, CHUNK], BF16, name="exsum")
                for pi in range(0, len(seq), 2):
                    pair = seq[pi : pi + 2]
                    sps = sc2_ps.tile([P, 2 * CHUNK], F32, name="sc2")
                    off = 0
                    packed = []
                    for jb, dt in pair:
                        qoff = (dt or 0) * P
                        n = CHUNK - qoff
                        nc.tensor.matmul(
                            sps[:, off : off + n],
                            kT[h][:, jb * P : (jb + 1) * P],
                            qTc[h][:, qoff:CHUNK],
                            start=True,
                            stop=True,
                            skip_group_check=True,
                        )
                        packed.append((jb, dt, qoff, n, off))
                        off += n
                    ext = exp_pool.tile([P, 2 * CHUNK], BF16, name="ex")
                    nc.scalar.activation(
                        ext[:, :off],
                        sps[:, :off],
                        mybir.ActivationFunctionType.Exp,
                        scale=scale,
                    )
                    if pair[0][1] is not None:
                        # diag pair: one packed triangular-mask multiply
                        m0 = 0 if pair[0][1] == 0 else 896
                        exm = exp_pool.tile([P, 2 * CHUNK], BF16, name="ex")
                        nc.vector.tensor_mul(
                            exm[:, :off], ext[:, :off], tri4_t[:, m0 : m0 + off]
                        )
                        ext = exm
                    for k_, (jb, dt, qoff, n, boff) in enumerate(packed):
                        idx = pi + k_
                        if idx == 0:
                            nc.vector.tensor_copy(es[:], ext[:, :CHUNK])
                        else:
                            nc.vector.tensor_add(
                                es[:, qoff:CHUNK],
                                es[:, qoff:CHUNK],
                                ext[:, boff : boff + n],
                            )
                        nc.tensor.matmul(
                            out_ps[:, qoff:CHUNK],
                            vnat[jb][:, h * P : (h + 1) * P],
                            ext[:, boff : boff + n],
                            start=(idx == 0),
                            stop=(idx == last),
                            skip_group_check=True,
                        )
                # proj matmuls of the previous chunk run before this head's
                # sum-matmul, giving the VectorE exsum chain time to drain.
                if prev_outT is not None:
                    emit_proj(prev_sc, h, prev_outT)
                sum_ps = mm_ps.tile([P, CHUNK], F32, name="mm")
                nc.tensor.matmul(sum_ps[:], ones_t[:], es[:], start=True, stop=True)
                rc = recip_pool.tile([P, CHUNK], F32, name="rc")
                nc.vector.reciprocal_approx_fast(rc[:], sum_ps[:])
                oT = outT_pool.tile([P, CHUNK], BF16, name=f"oT{h}")
                nc.vector.tensor_mul(oT[:], out_ps[:], rc[:])
                cur_outT.append(oT)
            prev_outT = cur_outT
            prev_sc = sc

        for sb_local in range(DIAG):
            emit_proj(SC - 1, sb_local, prev_outT)
    return nc


_NC = None
LAST_RESULTS = None


def _get_nc():
    global _NC
    if _NC is None:
        nc = bacc.Bacc(
            "TRN2", target_bir_lowering=False, debug=False, num_devices=N_CORES
        )
        _emit(nc)
        nc.compile()
        _NC = nc
    return _NC


def _core_inputs(hidden_states, c_attn_w, c_attn_b, c_proj_w, core):
    import ml_dtypes

    bf16 = ml_dtypes.bfloat16
    b, g = core // 4, core % 4
    h0 = H * g
    cols = []
    for part in range(3):
        for h in range(h0, h0 + H):
            base = part * E + h * P
            cols.extend(range(base, base + P))
    cols = np.asarray(cols)
    wqkv = np.ascontiguousarray(c_attn_w[:, cols]).astype(bf16)
    bq = np.ascontiguousarray(c_attn_b[cols]).astype(np.float32)
    bq = bq.reshape(NJ, P).T.copy()
    wproj = np.ascontiguousarray(c_proj_w[h0 * P : (h0 + H) * P, :]).astype(bf16)
    ii = np.arange(CHUNK)[None, :]
    pp = np.arange(P)[:, None]
    tri = (pp <= ii).astype(bf16)
    tri4 = np.concatenate(
        [tri[:, :512], tri[:, :384], tri[:, :256], tri[:, :128]], axis=1
    )
    ones = np.ones((P, P), dtype=bf16)
    xT = np.ascontiguousarray(hidden_states[b].astype(bf16).T)
    return {
        "xT": xT,
        "wqkv": wqkv,
        "bqkv": bq,
        "wproj": wproj,
        "tri4": tri4,
        "ones": ones,
    }


def kernel(hidden_states, c_attn_w, c_attn_b, c_proj_w, c_proj_b):
    global LAST_RESULTS
    hidden_states = np.asarray(hidden_states)
    c_attn_w = np.asarray(c_attn_w)
    c_attn_b = np.asarray(c_attn_b)
    c_proj_w = np.asarray(c_proj_w)
    c_proj_b = np.asarray(c_proj_b)

    nc = _get_nc()
    in_maps = [
        _core_inputs(hidden_states, c_attn_w, c_attn_b, c_proj_w, c)
        for c in range(N_CORES)
    ]
    res = run_bass_kernel_spmd(nc, in_maps, list(range(N_CORES)))
    LAST_RESULTS = res
    out = np.zeros((BATCH, S, E), dtype=np.float32)
    for c in range(N_CORES):
        out[c // 4] += res.results[c]["y"]
    out += c_proj_b.astype(np.float32)[None, None, :]
    return out
